# revision 1
# baseline (speedup 1.0000x reference)
"""Trainium2 Bass kernel for nn_CrossBaby_1 (B=32, S=128, V=8192, E=256).

Strategy (8 NeuronCores, single NEFF, collectives):
  - Step 1 (x @ w_emb.T, the 17 GFLOP matmul): data-parallel over batch.
    Each core computes hT for its 4 batches from a host-pretransposed,
    bf16-cast x shard. PSUM-accumulated over 64 K-chunks of V.
  - AllGather of hT (bf16, 256KB/core) + per-batch row sums s.
  - Steps 3-5 (w_red / w_red2, the 67MB of weights): tensor-parallel over
    the e/j feature dim — each core holds 1/8 of w_red and w_red2 and
    processes ALL 32 batches for its feature shard.
  - AllReduce of the partial y2 (32x256 f32).
  - Step 6 (w_out): tensor-parallel over vocab; each core emits
    out[:, c*1024:(c+1)*1024]; host concatenates.
  All matmul operands bf16 (fp32 PSUM accumulation); biases/activations fp32.
"""

import numpy as np
import ml_dtypes

B, S, V, E = 32, 128, 8192, 256
NC = 8
BL = B // NC    # 4 local batches
ES = E // NC    # 32 feature shard (steps 3-5)
VS = V // NC    # 1024 vocab shard (step 6)
NCOL = BL * S   # 512 columns of local hT
GHT = 2 * 128 * NCOL          # bf16 elements of hT in gather payload
GLEN = GHT + 128 * BL         # + flattened s

_CACHE: dict = {}


def _build_nc(reps: int = 1, stop_after: str = "all", skip_cc: bool = False,
              p1_bufs: int = 8, p1_mode: str = "resw"):
    import concourse.bacc as bacc
    import concourse.mybir as mybir
    import concourse.tile as tile

    bf = mybir.dt.bfloat16
    f32 = mybir.dt.float32
    AF = mybir.ActivationFunctionType
    ALU = mybir.AluOpType

    nc = bacc.Bacc("TRN2", target_bir_lowering=False, debug=False, num_devices=NC)

    xt = nc.dram_tensor("xt", [V, NCOL], bf, kind="ExternalInput")
    wembT = nc.dram_tensor("wembT", [V, E], bf, kind="ExternalInput")
    bemb = nc.dram_tensor("bemb", [E], f32, kind="ExternalInput")
    wrT = nc.dram_tensor("wrT", [ES, S, E], bf, kind="ExternalInput")
    bredrep = nc.dram_tensor("bredrep", [S, 16 * ES], f32, kind="ExternalInput")
    w2p = nc.dram_tensor("w2p", [ES, S, E], bf, kind="ExternalInput")
    bred2 = nc.dram_tensor("bred2", [E], f32, kind="ExternalInput")
    woT = nc.dram_tensor("woT", [E, VS], bf, kind="ExternalInput")
    boutrep = nc.dram_tensor("boutrep", [B, VS], f32, kind="ExternalInput")
    ones = nc.dram_tensor("ones", [S, 1], bf, kind="ExternalInput")
    ident = nc.dram_tensor("ident", [B, B], f32, kind="ExternalInput")
    out_ext = nc.dram_tensor("out", [B, VS], f32, kind="ExternalOutput")

    gin = nc.dram_tensor("gin", [GHT], bf)
    gout = nc.dram_tensor("gout", [NC, GHT], bf, addr_space="Shared")
    gin_s = nc.dram_tensor("gin_s", [S * BL], bf)
    gout_s = nc.dram_tensor("gout_s", [NC, S * BL], bf, addr_space="Shared")
    arin = nc.dram_tensor("arin", [B, E], f32)
    arout = nc.dram_tensor("arout", [B, E], f32, addr_space="Shared")

    groups = [list(range(NC))]

    with tile.TileContext(nc) as tc:
        with (
            tc.tile_pool(name="persist", bufs=1) as pp,
            tc.tile_pool(name="xload", bufs=p1_bufs) as xpool,
            tc.tile_pool(name="weload", bufs=p1_bufs) as wepool,
            tc.tile_pool(name="wrload", bufs=ES) as wrpool,
            tc.tile_pool(name="w2load", bufs=ES) as w2pool,
            tc.tile_pool(name="psum", bufs=1, space="PSUM") as psp,
        ):
            # ---------- persistent SBUF ----------
            hT_all = pp.tile([128, 2 * B * S], bf)       # [j128, (jc, b, s)]
            sT_all = pp.tile([128, B], bf)               # [k, (c,b)]
            weff = pp.tile([128, 2 * ES * B], bf)        # [j128, (jc, e, b)]
            y1 = pp.tile([128, B * ES], bf)              # [k, (b, j)]
            hsb = pp.tile([128, 2 * NCOL], bf)           # local hT [j128,(jc,n)]
            s_bf = pp.tile([1, NCOL], bf)
            bemb_sb = pp.tile([128, 2], f32)
            bredrep_sb = pp.tile([128, 16 * ES], f32)
            bred2_sb = pp.tile([128, 2], f32)
            ones_sb = pp.tile([128, 1], bf)
            ident_sb = pp.tile([B, B], f32)
            y2p_sb = pp.tile([B, E], f32)
            y2r_sb = pp.tile([B, E], f32)
            y2T = pp.tile([128, 2 * B], bf)              # [e128, (ec, b)]
            wo_sb = pp.tile([128, 2 * VS], bf)           # [e128, (ec, v)]
            boutrep_sb = pp.tile([B, VS], f32)
            outsb = pp.tile([B, VS], f32)

            nc.sync.dma_start(bemb_sb[:, :], bemb.ap().rearrange("(ec p) -> p ec", p=128))
            nc.sync.dma_start(bredrep_sb[:, :], bredrep[:, :])
            nc.sync.dma_start(bred2_sb[:, :], bred2.ap().rearrange("(ec p) -> p ec", p=128))
            nc.sync.dma_start(ones_sb[:, :], ones[:, :])
            nc.sync.dma_start(ident_sb[:, :], ident[:, :])
            nc.sync.dma_start(boutrep_sb[:, :], boutrep[:, :])
            nc.sync.dma_start(
                wo_sb.rearrange("p (ec v) -> p ec v", ec=2),
                woT.ap().rearrange("(ec p) v -> p ec v", p=128),
            )

            weff_v = weff.rearrange("p (jc e b) -> p jc e b", jc=2, e=ES)
            y1_v = y1.rearrange("p (b j) -> p b j", b=B)

            _ORD = ["p1", "gather", "p3", "p4", "p5", "ar", "all"]

            def upto(stage):
                return _ORD.index(stage) <= _ORD.index(stop_after)

            for _rep in range(reps):
                # ------- phase 1: hT = relu(w_embT.T @ xT + b_emb) -------
                ph0 = psp.tile([128, NCOL], f32, tag="ph0", name="ph0")
                ph1 = psp.tile([128, NCOL], f32, tag="ph1", name="ph1")
                ps = psp.tile([1, NCOL], f32, tag="ps", name="ps")
                phs = [ph0, ph1]
                NV = V // 128
                if p1_mode == "resw":
                    # resident w_embT: one [128, 2*E] tile per v-chunk group,
                    # loaded up front in NW_G chunks
                    wemb_res = pp.tile([128, NV * E], bf, name=f"wemb_res{_rep}",
                                       tag="wemb_res")
                    wemb_res_v = wemb_res.rearrange("p (vc e) -> p vc e", vc=NV)
                    for g in range(8):
                        nc.sync.dma_start(
                            wemb_res_v[:, g * 8:(g + 1) * 8, :],
                            wembT.ap().rearrange("(vc p) e -> p vc e", p=128)[
                                :, g * 8:(g + 1) * 8, :
                            ],
                        )
                XG = 2 if p1_mode in ("bigx", "resw") else 1
                for vco in range(NV // XG):
                    xt_t = xpool.tile([128, XG, NCOL], bf, tag="xt", name="xt_t")
                    nc.sync.dma_start(
                        xt_t[:, :, :],
                        xt.ap().rearrange("(vc p) n -> p vc n", p=128)[
                            :, vco * XG:(vco + 1) * XG, :
                        ],
                    )
                    for vci in range(XG):
                        vc = vco * XG + vci
                        if p1_mode == "resw":
                            we_t = wemb_res_v[:, vc, :]
                        else:
                            we_tt = wepool.tile([128, E], bf, tag="we", name="we_t")
                            nc.sync.dma_start(
                                we_tt[:, :], wembT[vc * 128:(vc + 1) * 128, :]
                            )
                            we_t = we_tt
                        for ec in range(2):
                            nc.tensor.matmul(
                                phs[ec][:, :],
                                we_t[:, ec * 128:(ec + 1) * 128],
                                xt_t[:, vci, :],
                                start=(vc == 0),
                                stop=(vc == NV - 1),
                            )
                for ec in range(2):
                    nc.scalar.activation(
                        hsb[:, ec * NCOL:(ec + 1) * NCOL],
                        phs[ec][:, :],
                        AF.Relu,
                        bias=bemb_sb[:, ec:ec + 1],
                    )
                # s = column sums of hT (over both j-chunks)
                for ec in range(2):
                    nc.tensor.matmul(
                        ps[:, :],
                        ones_sb[:, 0:1],
                        hsb[:, ec * NCOL:(ec + 1) * NCOL],
                        start=(ec == 0),
                        stop=(ec == 1),
                    )
                nc.vector.tensor_copy(s_bf[:, :], ps[:, :])

                if not upto("gather"):
                    nc.gpsimd.dma_start(out_ext[:, 0:NCOL], hsb[0:B, 0:NCOL])
                    continue
                # ------- gather s (tiny, first) then hT across cores -------
                nc.sync.dma_start(
                    gin_s.ap().rearrange("(one n) -> one n", one=1),
                    s_bf[:, :],
                )
                nc.sync.dma_start(
                    gin.ap().rearrange("(jc p n) -> p jc n", jc=2, p=128),
                    hsb.rearrange("p (jc n) -> p jc n", jc=2),
                )
                if skip_cc:
                    nc.sync.dma_start(gout_s.ap()[0], gin_s.ap()[:])
                    nc.sync.dma_start(gout.ap()[0], gin.ap()[:])
                else:
                    nc.gpsimd.collective_compute(
                        "AllGather", ALU.bypass, groups,
                        ins=[gin_s.ap().opt()], outs=[gout_s.ap().opt()],
                    )
                    nc.gpsimd.collective_compute(
                        "AllGather", ALU.bypass, groups,
                        ins=[gin.ap().opt()], outs=[gout.ap().opt()],
                    )
                for c in range(NC):
                    nc.sync.dma_start(
                        sT_all[:, c * BL:(c + 1) * BL],
                        gout_s.ap()[c].rearrange("(b k) -> k b", b=BL),
                    )
                for jc in range(2):
                    nc.sync.dma_start(
                        hT_all[:, jc * B * S:(jc + 1) * B * S].rearrange(
                            "p (c n) -> p c n", c=NC
                        ),
                        gout.ap()[:, jc * 128 * NCOL:(jc + 1) * 128 * NCOL].rearrange(
                            "c (p n) -> p c n", p=128
                        ),
                    )

                if not upto("p3"):
                    nc.gpsimd.dma_start(out_ext[:, 0:NCOL], hT_all[0:B, 0:NCOL])
                    nc.gpsimd.dma_start(out_ext[:, NCOL:NCOL + B], sT_all[0:B, :])
                    continue
                # ------- step 3: Weff[b, e, j] = sum_k Wr[e,k,j] s[b,k] -------
                # 8 e-values per PSUM bank; one batched copy per bank.
                weff_p = weff.rearrange("p (jc e b) -> p e jc b", jc=2, e=ES)
                for g in range(ES // 8):
                    pw8 = psp.tile([128, 512], f32, tag="p32", bufs=2, name="pw8")
                    for e8 in range(8):
                        el = g * 8 + e8
                        wr_t = wrpool.tile([128, E], bf, tag="wr", name="wr_t")
                        nc.sync.dma_start(wr_t[:, :], wrT.ap()[el])
                        for jc in range(2):
                            off = e8 * 64 + jc * 32
                            nc.tensor.matmul(
                                pw8[:, off:off + 32],
                                wr_t[:, jc * 128:(jc + 1) * 128],
                                sT_all[:, :],
                            )
                    nc.vector.tensor_copy(
                        weff_p[:, g * 8:(g + 1) * 8, :, :],
                        pw8.rearrange("p (e jc b) -> p e jc b", e=8, jc=2),
                    )

                if not upto("p4"):
                    nc.gpsimd.dma_start(out_ext[:, 0:64], weff[0:B, 0:64])
                    nc.gpsimd.dma_start(out_ext[:, 64:64 + NCOL], hT_all[0:B, 0:NCOL])
                    continue
                # ------- step 4: y1 = relu(h @ Weff_b^T + b_red) -------
                # 16 batches per PSUM bank; one add+relu pair per bank.
                for g in range(B // 16):
                    py16 = psp.tile([128, 512], f32, tag="p32", bufs=2, name="py16")
                    for bl in range(16):
                        b = g * 16 + bl
                        for jc in range(2):
                            nc.tensor.matmul(
                                py16[:, bl * ES:(bl + 1) * ES],
                                hT_all[:, jc * B * S + b * S: jc * B * S + (b + 1) * S],
                                weff_v[:, jc, :, b],
                                start=(jc == 0),
                                stop=(jc == 1),
                            )
                    nc.vector.tensor_tensor(py16[:, :], py16[:, :], bredrep_sb[:, :], ALU.add)
                    nc.scalar.activation(
                        y1[:, g * 512:(g + 1) * 512], py16[:, :], AF.Relu
                    )

                if not upto("p5"):
                    nc.gpsimd.dma_start(out_ext[:, 0:64], y1[0:B, 0:64])
                    continue
                # ------- step 5: y2p[b, eo] = sum_{k, j in shard} -------
                py2 = psp.tile([B, E], f32, tag="py2", name="py2")
                for jl in range(ES):
                    w2_t = w2pool.tile([128, E], bf, tag="w2", name="w2_t")
                    nc.sync.dma_start(w2_t[:, :], w2p.ap()[jl])
                    nc.tensor.matmul(
                        py2[:, :],
                        y1_v[:, :, jl],
                        w2_t[:, :],
                        start=(jl == 0),
                        stop=(jl == ES - 1),
                    )
                nc.vector.tensor_copy(y2p_sb[:, :], py2[:, :])

                if not upto("ar"):
                    nc.sync.dma_start(out_ext[:, 0:E], y2p_sb[:, :])
                    continue
                # ------- all-reduce partial y2 -------
                nc.sync.dma_start(arin[:, :], y2p_sb[:, :])
                if skip_cc:
                    nc.sync.dma_start(arout[:, :], arin[:, :])
                else:
                    nc.gpsimd.collective_compute(
                        "AllReduce", ALU.add, groups,
                        ins=[arin.ap().opt()], outs=[arout.ap().opt()],
                    )
                nc.sync.dma_start(y2r_sb[:, :], arout[:, :])

                # ------- y2T = relu(y2 + b_red2)^T -------
                for ec in range(2):
                    pst = psp.tile([128, B], f32, tag="p32", bufs=2, name="pst")
                    nc.tensor.transpose(
                        pst[:, :], y2r_sb[:, ec * 128:(ec + 1) * 128], ident_sb[:, :]
                    )
                    nc.scalar.activation(
                        y2T[:, ec * B:(ec + 1) * B],
                        pst[:, :],
                        AF.Relu,
                        bias=bred2_sb[:, ec:ec + 1],
                    )

                # ------- step 6: out = y2 @ w_out^T + b_out -------
                for nv in range(2):
                    pso = psp.tile([B, 512], f32, tag="po", bufs=2, name="pso")
                    for ec in range(2):
                        nc.tensor.matmul(
                            pso[:, :],
                            y2T[:, ec * B:(ec + 1) * B],
                            wo_sb[:, ec * VS + nv * 512: ec * VS + (nv + 1) * 512],
                            start=(ec == 0),
                            stop=(ec == 1),
                        )
                    nc.vector.tensor_tensor(
                        outsb[:, nv * 512:(nv + 1) * 512],
                        pso[:, :],
                        boutrep_sb[:, nv * 512:(nv + 1) * 512],
                        ALU.add,
                    )
                nc.sync.dma_start(out_ext[:, :], outsb[:, :])

    nc.compile()
    return nc


def _get_nc():
    if "nc" not in _CACHE:
        _CACHE["nc"] = _build_nc()
    return _CACHE["nc"]


def _pack_inputs(x, w_emb, b_emb, w_red, b_red, w_red2, b_red2, w_out, b_out):
    bf = ml_dtypes.bfloat16
    f32 = np.float32
    wembT = np.ascontiguousarray(w_emb.T).astype(bf)                 # [V, E]
    Wr = np.asarray(w_red).reshape(E, S, E)                          # [e, k, j]
    W2 = np.asarray(w_red2).reshape(E, S, E)                         # [eo, k, j]
    woT = np.ascontiguousarray(np.asarray(w_out).T)                  # [E, V]
    bemb = np.ascontiguousarray(b_emb).astype(f32)
    bred2 = np.ascontiguousarray(b_red2).astype(f32)
    ones = np.ones((S, 1), dtype=bf)
    ident = np.eye(B, dtype=f32)

    in_maps = []
    for c in range(NC):
        xs = np.asarray(x[c * BL:(c + 1) * BL])                      # [4, S, V]
        xt = np.ascontiguousarray(xs.transpose(2, 0, 1).reshape(V, NCOL)).astype(bf)
        wrT_c = np.ascontiguousarray(Wr[c * ES:(c + 1) * ES]).astype(bf)      # [el,k,j]
        w2p_c = np.ascontiguousarray(
            W2[:, :, c * ES:(c + 1) * ES].transpose(2, 1, 0)
        ).astype(bf)                                                  # [jl,k,eo]
        woT_c = np.ascontiguousarray(woT[:, c * VS:(c + 1) * VS]).astype(bf)  # [E,VS]
        bredrep = np.ascontiguousarray(
            np.broadcast_to(np.tile(b_red[c * ES:(c + 1) * ES], 16), (S, 16 * ES))
        ).astype(f32)
        boutrep = np.ascontiguousarray(
            np.broadcast_to(b_out[c * VS:(c + 1) * VS], (B, VS))
        ).astype(f32)
        in_maps.append({
            "xt": xt, "wembT": wembT, "bemb": bemb,
            "wrT": wrT_c, "bredrep": bredrep,
            "w2p": w2p_c, "bred2": bred2,
            "woT": woT_c, "boutrep": boutrep,
            "ones": ones, "ident": ident,
        })
    return in_maps


def kernel(x, w_emb, b_emb, w_red, b_red, w_red2, b_red2, w_out, b_out):
    from concourse.bass_utils import run_bass_kernel_spmd

    nc = _get_nc()
    x, w_emb, b_emb, w_red, b_red, w_red2, b_red2, w_out, b_out = (
        np.asarray(a, dtype=np.float32)
        for a in (x, w_emb, b_emb, w_red, b_red, w_red2, b_red2, w_out, b_out)
    )
    in_maps = _pack_inputs(x, w_emb, b_emb, w_red, b_red, w_red2, b_red2, w_out, b_out)
    res = run_bass_kernel_spmd(nc, in_maps, core_ids=list(range(NC)))
    out = np.concatenate([res.results[c]["out"] for c in range(NC)], axis=1)
    return np.ascontiguousarray(out, dtype=np.float32)



# revision 16
# speedup vs baseline: 1.3392x; 1.3392x over previous
"""Trainium2 Bass kernel for nn_CrossBaby_1 (B=32, S=128, V=8192, E=256).

Strategy (8 NeuronCores, single NEFF, collectives):
  - Step 1 (x @ w_emb.T, 17 GFLOP): data-parallel over batch. Each core
    computes hT for its 4 batches from a host-pretransposed, centered
    (x-0.5) shard in fp8-e3m4 (the 0.5*sum(w_emb) correction is folded
    into the bias on host). w_emb stays bf16. x and w_emb stream
    just-in-time on the two HWDGE rings (sync=x, scalar=w_emb) in ~0.5MB
    chunks so the PE starts after the first chunk.
  - ONE merged AllGather of (hT bf16 + row-sums s) across the 8 cores.
  - Steps 3-5 (w_red / w_red2, 67MB of weights): tensor-parallel over
    the e/j feature dim; weight shards stream on the scalar ring behind
    w_emb so they arrive during the gather window.
  - y2 partial reduction: AllGather (cheaper floor than AllReduce) +
    local vector-tree sum.
  - Step 6 (w_out): tensor-parallel over vocab; host concatenates.
  All matmuls accumulate in fp32 PSUM; weights/activations bf16.
"""

import numpy as np
import ml_dtypes

B, S, V, E = 32, 128, 8192, 256
NC = 8
BL = B // NC    # 4 local batches
ES = E // NC    # 32 feature shard (steps 3-5)
VS = V // NC    # 1024 vocab shard (step 6)
NCOL = BL * S   # 512 columns of local hT
NV = V // 128   # 64 v-chunks
GHT = 2 * 128 * NCOL          # bf16 elements of hT in gather payload
GLEN = GHT + NCOL             # + flattened s row

X_FP8 = True    # x shard in fp8-e3m4 (centered); False -> bf16
XCH = 8         # v-chunks per x/w_emb DMA chunk (8 -> 0.5MB fp8 x chunks)

_CACHE: dict = {}


def _build_nc(reps: int = 1, stop_after: str = "all", skip_cc: bool = False):
    import concourse.bacc as bacc
    import concourse.mybir as mybir
    import concourse.tile as tile

    bf = mybir.dt.bfloat16
    f32 = mybir.dt.float32
    xdt = mybir.dt.float8e3 if X_FP8 else bf
    AF = mybir.ActivationFunctionType
    ALU = mybir.AluOpType

    nc = bacc.Bacc("TRN2", target_bir_lowering=False, debug=False, num_devices=NC)

    xt = nc.dram_tensor("xt", [128, NV * NCOL], xdt, kind="ExternalInput")
    wemb = nc.dram_tensor("wemb", [128, NV * E], bf, kind="ExternalInput")
    bembe = nc.dram_tensor("bembe", [128, 2], f32, kind="ExternalInput")
    wr = nc.dram_tensor("wr", [128, ES * E], bf, kind="ExternalInput")
    bredrep = nc.dram_tensor("bredrep", [128, 16 * ES], f32, kind="ExternalInput")
    w2 = nc.dram_tensor("w2", [128, ES * E], bf, kind="ExternalInput")
    bred2 = nc.dram_tensor("bred2", [128, 2], f32, kind="ExternalInput")
    wo = nc.dram_tensor("wo", [128, 2 * VS], bf, kind="ExternalInput")
    boutrep = nc.dram_tensor("boutrep", [B, VS], f32, kind="ExternalInput")
    ones = nc.dram_tensor("ones", [128, 1], bf, kind="ExternalInput")
    out_ext = nc.dram_tensor("out", [B, VS], f32, kind="ExternalOutput")

    gin = nc.dram_tensor("gin", [GLEN], bf)
    gout = nc.dram_tensor("gout", [NC, GLEN], bf, addr_space="Shared")
    y2g_in = nc.dram_tensor("y2g_in", [B, E], f32)
    y2g_out = nc.dram_tensor("y2g_out", [NC * B, E], f32, addr_space="Shared")

    groups = [list(range(NC))]

    with tile.TileContext(nc) as tc:
        with (
            tc.tile_pool(name="persist", bufs=1) as pp,
            tc.tile_pool(name="xload", bufs=3) as xpool,
            tc.tile_pool(name="psum", bufs=1, space="PSUM") as psp,
        ):
            # ---------- persistent SBUF ----------
            wemb_res = pp.tile([128, NV * E], bf)        # [v128, (vc, e)]
            wr_res = pp.tile([128, ES * E], bf)          # [k, (el, j)]
            w2_res = pp.tile([128, ES * E], bf)          # [k, (jl, eo)]
            wo_sb = pp.tile([128, 2 * VS], bf)           # [e128, (ec, v)]
            hT_all = pp.tile([128, 2 * B * S], bf)       # [j128, (jc, b, s)]
            sT_all = pp.tile([128, B], bf)               # [k, (c, bl)]
            weff = pp.tile([128, 2 * ES * B], bf)        # [j128, (jc, e, b)]
            y1 = pp.tile([128, B * ES], bf)              # [k, (b, j)]
            hsb = pp.tile([128, 2 * NCOL], bf)           # local hT [j128, (jc, n)]
            s_bf = pp.tile([1, NCOL], bf)
            bembe_sb = pp.tile([128, 2], f32)
            bredrep_sb = pp.tile([128, 16 * ES], f32)
            bred2_sb = pp.tile([128, 2], f32)
            ones_sb = pp.tile([128, 1], bf)
            y2sum = pp.tile([B, NC * E], f32)            # [b, (c, e)]
            y2Tf = pp.tile([128, 2 * B], f32)            # [e128, (ec, b)]
            y2T = pp.tile([128, 2 * B], bf)
            boutrep_sb = pp.tile([B, VS], f32)
            outsb = pp.tile([B, VS], f32)

            # tiny biases first on the scalar (ACT) HWDGE ring
            nc.scalar.dma_start(bembe_sb[:, :], bembe[:, :])
            nc.scalar.dma_start(bred2_sb[:, :], bred2[:, :])
            nc.scalar.dma_start(ones_sb[:, :], ones[:, :])

            weff_v = weff.rearrange("p (jc e b) -> p jc e b", jc=2, e=ES)
            weff_p = weff.rearrange("p (jc e b) -> p e jc b", jc=2, e=ES)
            y1_v = y1.rearrange("p (b j) -> p b j", b=B)

            _ORD = ["p1", "gather", "p3", "p4", "p5", "ar", "tail1", "tail2", "all"]

            def upto(stage):
                return _ORD.index(stage) <= _ORD.index(stop_after)

            for _rep in range(reps):
                # ------- phase 1: hT = relu(w_embT.T @ (x-.5)T + bembe) -------
                ph0 = psp.tile([128, NCOL], f32, tag="ph0", name="ph0")
                ph1 = psp.tile([128, NCOL], f32, tag="ph1", name="ph1")
                ps = psp.tile([1, NCOL], f32, tag="ps", name="ps")
                phs = [ph0, ph1]
                for g in range(NV // XCH):
                    xt_t = xpool.tile([128, XCH * NCOL], xdt, tag="xt", name="xt_t")
                    nc.sync.dma_start(
                        xt_t[:, :], xt[:, g * XCH * NCOL:(g + 1) * XCH * NCOL]
                    )
                    nc.scalar.dma_start(
                        wemb_res[:, g * XCH * E:(g + 1) * XCH * E],
                        wemb[:, g * XCH * E:(g + 1) * XCH * E],
                    )
                    for vci in range(XCH):
                        vc = g * XCH + vci
                        for ec in range(2):
                            nc.tensor.matmul(
                                phs[ec][:, :],
                                wemb_res[:, vc * E + ec * 128: vc * E + (ec + 1) * 128],
                                xt_t[:, vci * NCOL:(vci + 1) * NCOL],
                                start=(vc == 0),
                                stop=(vc == NV - 1),
                            )
                for ec in range(2):
                    nc.scalar.activation(
                        hsb[:, ec * NCOL:(ec + 1) * NCOL],
                        phs[ec][:, :],
                        AF.Relu,
                        bias=bembe_sb[:, ec:ec + 1],
                    )
                # s = column sums of hT (over both j-chunks)
                for ec in range(2):
                    nc.tensor.matmul(
                        ps[:, :],
                        ones_sb[:, 0:1],
                        hsb[:, ec * NCOL:(ec + 1) * NCOL],
                        start=(ec == 0),
                        stop=(ec == 1),
                    )
                nc.vector.tensor_copy(s_bf[:, :], ps[:, :])

                if not upto("gather"):
                    nc.gpsimd.dma_start(out_ext[:, 0:NCOL], hsb[0:B, 0:NCOL])
                    continue
                # ------- single merged AllGather of (hT, s) -------
                nc.sync.dma_start(
                    gin.ap()[0:GHT].rearrange("(jc p n) -> p jc n", jc=2, p=128),
                    hsb.rearrange("p (jc n) -> p jc n", jc=2),
                )
                nc.sync.dma_start(
                    gin.ap()[GHT:GLEN].rearrange("(one n) -> one n", one=1),
                    s_bf[:, :],
                )
                if skip_cc:
                    nc.sync.dma_start(gout.ap()[0], gin.ap()[:])
                else:
                    nc.gpsimd.collective_compute(
                        "AllGather", ALU.bypass, groups,
                        ins=[gin.ap().opt()], outs=[gout.ap().opt()],
                    )
                # steps 3-5 weights stream on the ACT ring during the gather
                nc.scalar.dma_start(wr_res[:, 0:ES * E // 2], wr[:, 0:ES * E // 2])
                nc.scalar.dma_start(wr_res[:, ES * E // 2:], wr[:, ES * E // 2:])
                nc.scalar.dma_start(bredrep_sb[:, :], bredrep[:, :])
                nc.scalar.dma_start(w2_res[:, 0:ES * E // 2], w2[:, 0:ES * E // 2])
                nc.scalar.dma_start(w2_res[:, ES * E // 2:], w2[:, ES * E // 2:])
                nc.scalar.dma_start(
                    wo_sb.rearrange("p (ec v) -> p ec v", ec=2),
                    wo.ap().rearrange("p (ec v) -> p ec v", ec=2),
                )
                nc.scalar.dma_start(boutrep_sb[:, :], boutrep[:, :])

                # readback: hT of all cores + s rows
                for jc in range(2):
                    nc.sync.dma_start(
                        hT_all[:, jc * B * S:(jc + 1) * B * S].rearrange(
                            "p (c n) -> p c n", c=NC
                        ),
                        gout.ap()[:, jc * 128 * NCOL:(jc + 1) * 128 * NCOL].rearrange(
                            "c (p n) -> p c n", p=128
                        ),
                    )
                sT_v = sT_all.rearrange("p (c bl) -> p c bl", c=NC)
                g_s = gout.ap()[:, GHT:GLEN].rearrange("c (bl k) -> k c bl", bl=BL)
                for bl in range(BL):
                    nc.sync.dma_start(sT_v[:, :, bl], g_s[:, :, bl])

                if not upto("p3"):
                    nc.gpsimd.dma_start(out_ext[:, 0:NCOL], hT_all[0:B, 0:NCOL])
                    nc.gpsimd.dma_start(out_ext[:, NCOL:NCOL + B], sT_all[0:B, :])
                    continue
                # ------- step 3: Weff[b, e, j] = sum_k Wr[e,k,j] s[b,k] -------
                for g in range(ES // 8):
                    pw8 = psp.tile([128, 512], f32, tag="p32", bufs=2, name="pw8")
                    for e8 in range(8):
                        el = g * 8 + e8
                        for jc in range(2):
                            off = e8 * 64 + jc * 32
                            nc.tensor.matmul(
                                pw8[:, off:off + 32],
                                wr_res[:, el * E + jc * 128: el * E + (jc + 1) * 128],
                                sT_all[:, :],
                            )
                    nc.vector.tensor_copy(
                        weff_p[:, g * 8:(g + 1) * 8, :, :],
                        pw8.rearrange("p (e jc b) -> p e jc b", e=8, jc=2),
                    )

                if not upto("p4"):
                    nc.gpsimd.dma_start(out_ext[:, 0:64], weff[0:B, 0:64])
                    nc.gpsimd.dma_start(out_ext[:, 64:64 + NCOL], hT_all[0:B, 0:NCOL])
                    continue
                # ------- step 4: y1 = relu(h @ Weff_b^T + b_red) -------
                for g in range(B // 16):
                    py16 = psp.tile([128, 512], f32, tag="p32", bufs=2, name="py16")
                    for bl in range(16):
                        b = g * 16 + bl
                        for jc in range(2):
                            nc.tensor.matmul(
                                py16[:, bl * ES:(bl + 1) * ES],
                                hT_all[:, jc * B * S + b * S: jc * B * S + (b + 1) * S],
                                weff_v[:, jc, :, b],
                                start=(jc == 0),
                                stop=(jc == 1),
                            )
                    nc.vector.tensor_tensor(py16[:, :], py16[:, :], bredrep_sb[:, :], ALU.add)
                    nc.scalar.activation(
                        y1[:, g * 512:(g + 1) * 512], py16[:, :], AF.Relu
                    )

                if not upto("p5"):
                    nc.gpsimd.dma_start(out_ext[:, 0:64], y1[0:B, 0:64])
                    continue
                # ------- step 5: y2p[b, eo] = sum_{k, j in shard} -------
                py2 = psp.tile([B, E], f32, tag="py2", name="py2")
                for jl in range(ES):
                    nc.tensor.matmul(
                        py2[:, :],
                        y1_v[:, :, jl],
                        w2_res[:, jl * E:(jl + 1) * E],
                        start=(jl == 0),
                        stop=(jl == ES - 1),
                    )
                nc.vector.tensor_copy(y2sum[:, 0:E], py2[:, :])

                if not upto("ar"):
                    nc.sync.dma_start(out_ext[:, 0:E], y2sum[:, 0:E])
                    continue
                # ------- AllGather partial y2, local tree-sum -------
                nc.sync.dma_start(y2g_in[:, :], y2sum[:, 0:E])
                if skip_cc:
                    for c in range(NC):
                        nc.sync.dma_start(y2g_out.ap()[c * B:(c + 1) * B], y2g_in[:, :])
                else:
                    nc.gpsimd.collective_compute(
                        "AllGather", ALU.bypass, groups,
                        ins=[y2g_in.ap().opt()], outs=[y2g_out.ap().opt()],
                    )
                nc.sync.dma_start(
                    y2sum.rearrange("b (c e) -> b c e", c=NC),
                    y2g_out.ap().rearrange("(c b) e -> b c e", c=NC),
                )
                for half in (4, 2, 1):
                    nc.vector.tensor_tensor(
                        y2sum[:, 0:half * E],
                        y2sum[:, 0:half * E],
                        y2sum[:, half * E:2 * half * E],
                        ALU.add,
                    )

                if not upto("tail1"):
                    nc.gpsimd.dma_start(out_ext[:, 0:E], y2sum[:, 0:E])
                    continue

                # ------- y2T = relu(y2 + b_red2)^T (DVE 32-blocks) -------
                for ec in range(2):
                    for j4 in range(4):
                        nc.vector.transpose(
                            y2Tf[j4 * 32:(j4 + 1) * 32, ec * B:(ec + 1) * B],
                            y2sum[:, ec * 128 + j4 * 32: ec * 128 + (j4 + 1) * 32],
                        )
                    nc.scalar.activation(
                        y2T[:, ec * B:(ec + 1) * B],
                        y2Tf[:, ec * B:(ec + 1) * B],
                        AF.Relu,
                        bias=bred2_sb[:, ec:ec + 1],
                    )

                if not upto("tail2"):
                    nc.gpsimd.dma_start(out_ext[:, 0:2 * B], y2T[0:B, :])
                    continue
                # ------- step 6: out = y2 @ w_out^T + b_out -------
                for nv in range(2):
                    pso = psp.tile([B, 512], f32, tag="po", bufs=2, name="pso")
                    for ec in range(2):
                        nc.tensor.matmul(
                            pso[:, :],
                            y2T[:, ec * B:(ec + 1) * B],
                            wo_sb[:, ec * VS + nv * 512: ec * VS + (nv + 1) * 512],
                            start=(ec == 0),
                            stop=(ec == 1),
                        )
                    nc.vector.tensor_tensor(
                        outsb[:, nv * 512:(nv + 1) * 512],
                        pso[:, :],
                        boutrep_sb[:, nv * 512:(nv + 1) * 512],
                        ALU.add,
                    )
                nc.sync.dma_start(out_ext[:, :], outsb[:, :])

    nc.compile()
    return nc


def _get_nc():
    if "nc" not in _CACHE:
        _CACHE["nc"] = _build_nc()
    return _CACHE["nc"]


def _pm(a):
    """[V-like rows, cols] -> partition-major [128, (chunks, cols)]."""
    v, c = a.shape
    return np.ascontiguousarray(
        a.reshape(v // 128, 128, c).transpose(1, 0, 2).reshape(128, -1)
    )


def _pack_inputs(x, w_emb, b_emb, w_red, b_red, w_red2, b_red2, w_out, b_out):
    bf = ml_dtypes.bfloat16
    xdt = ml_dtypes.float8_e3m4 if X_FP8 else bf
    f32 = np.float32

    x = np.asarray(x, f32)
    w_emb = np.asarray(w_emb, f32)
    wembT = _pm(np.ascontiguousarray(w_emb.T)).astype(bf)            # [128,(vc,e)]
    bemb_eff = (np.asarray(b_emb, np.float64)
                + 0.5 * np.asarray(w_emb, np.float64).sum(axis=1)).astype(f32)
    bembe = np.ascontiguousarray(bemb_eff.reshape(2, 128).T)         # [128, 2]
    Wr = np.asarray(w_red, f32).reshape(E, S, E)                     # [e, k, j]
    W2 = np.asarray(w_red2, f32).reshape(E, S, E)                    # [eo, k, j]
    woT = np.ascontiguousarray(np.asarray(w_out, f32).T)             # [E, V]
    bred2c = np.ascontiguousarray(
        np.asarray(b_red2, f32).reshape(2, 128).T)                   # [128, 2]
    ones = np.ones((128, 1), dtype=bf)

    in_maps = []
    for c in range(NC):
        xs = np.asarray(x[c * BL:(c + 1) * BL])                      # [4, S, V]
        xc = xs.transpose(2, 0, 1).reshape(V, NCOL) - 0.5            # [V, 512]
        xt = _pm(xc).astype(xdt)                                     # [128,(vc,n)]
        # wr: [k, (el, j)]
        wr_c = np.ascontiguousarray(
            Wr[c * ES:(c + 1) * ES].transpose(1, 0, 2).reshape(S, ES * E)
        ).astype(bf)
        # w2: [k, (jl, eo)]
        w2_c = np.ascontiguousarray(
            W2[:, :, c * ES:(c + 1) * ES].transpose(1, 2, 0).reshape(S, ES * E)
        ).astype(bf)
        # wo: [e128, (ec, v)]
        wo_c = _pm(woT[:, c * VS:(c + 1) * VS]).astype(bf)           # [128,(ec,v)]
        bredrep = np.ascontiguousarray(
            np.broadcast_to(np.tile(b_red[c * ES:(c + 1) * ES], 16), (S, 16 * ES))
        ).astype(f32)
        boutrep = np.ascontiguousarray(
            np.broadcast_to(b_out[c * VS:(c + 1) * VS], (B, VS))
        ).astype(f32)
        in_maps.append({
            "xt": xt, "wemb": wembT, "bembe": bembe,
            "wr": wr_c, "bredrep": bredrep,
            "w2": w2_c, "bred2": bred2c,
            "wo": wo_c, "boutrep": boutrep,
            "ones": ones,
        })
    return in_maps


def kernel(x, w_emb, b_emb, w_red, b_red, w_red2, b_red2, w_out, b_out):
    from concourse.bass_utils import run_bass_kernel_spmd

    nc = _get_nc()
    x, w_emb, b_emb, w_red, b_red, w_red2, b_red2, w_out, b_out = (
        np.asarray(a, dtype=np.float32)
        for a in (x, w_emb, b_emb, w_red, b_red, w_red2, b_red2, w_out, b_out)
    )
    in_maps = _pack_inputs(x, w_emb, b_emb, w_red, b_red, w_red2, b_red2, w_out, b_out)
    res = run_bass_kernel_spmd(nc, in_maps, core_ids=list(range(NC)))
    out = np.concatenate([res.results[c]["out"] for c in range(NC)], axis=1)
    return np.ascontiguousarray(out, dtype=np.float32)


# revision 30
# speedup vs baseline: 2.0460x; 1.5278x over previous
"""Trainium2 Bass kernel for nn_CrossBaby_1 (B=32, S=128, V=8192, E=256).

Strategy (8 NeuronCores, single NEFF, collectives):
  - Step 1 (x @ w_emb.T, 17 GFLOP): data-parallel over batch. Each core
    computes hT for its 4 batches from a host-pretransposed, centered
    (x-0.5) shard in fp8-e3m4 (the 0.5*sum(w_emb) correction is folded
    into the bias on host). w_emb stays bf16. x and w_emb stream
    just-in-time on the two HWDGE rings (sync=x, scalar=w_emb) in ~0.5MB
    chunks so the PE starts after the first chunk.
  - ONE merged AllGather of (hT bf16 + row-sums s) across the 8 cores.
  - Steps 3-5 (w_red / w_red2, 67MB of weights): tensor-parallel over
    the e/j feature dim; weight shards stream on the scalar ring behind
    w_emb so they arrive during the gather window.
  - y2 partial reduction: AllGather (cheaper floor than AllReduce) +
    local vector-tree sum.
  - Step 6 (w_out): tensor-parallel over vocab; host concatenates.
  All matmuls accumulate in fp32 PSUM; weights/activations bf16.
"""

import numpy as np
import ml_dtypes

B, S, V, E = 32, 128, 8192, 256
NC = 8
BL = B // NC    # 4 local batches
ES = E // NC    # 32 feature shard (steps 3-5)
VS = V // NC    # 1024 vocab shard (step 6)
NCOL = BL * S   # 512 columns of local hT
NV = V // 128   # 64 v-chunks
GHT = 2 * 128 * NCOL          # bf16 elements of hT in gather payload
GLEN = GHT + NCOL             # + flattened s row

X_FP8 = True    # x shard in fp8-e3m4 (centered); False -> bf16
H_FP8 = False   # fp8 gather payload: disabled — e3m4 h costs ~2e-2 end-to-end
XCH = 8         # v-chunks per x/w_emb DMA chunk (8 -> 0.5MB fp8 x chunks)

_CACHE: dict = {}


def _build_nc(reps: int = 1, stop_after: str = "all", skip_cc: bool = False):
    import concourse.bacc as bacc
    import concourse.mybir as mybir
    import concourse.tile as tile

    bf = mybir.dt.bfloat16
    f32 = mybir.dt.float32
    xdt = mybir.dt.float8e3 if X_FP8 else bf
    hdt = mybir.dt.float8e3 if H_FP8 else bf
    AF = mybir.ActivationFunctionType
    ALU = mybir.AluOpType

    nc = bacc.Bacc("TRN2", target_bir_lowering=False, debug=False, num_devices=NC)

    xt = nc.dram_tensor("xt", [128, NV * NCOL], xdt, kind="ExternalInput")
    wemb = nc.dram_tensor("wemb", [128, NV * E], bf, kind="ExternalInput")
    bembe = nc.dram_tensor("bembe", [128, 2], f32, kind="ExternalInput")
    wr = nc.dram_tensor("wr", [128, ES * E], bf, kind="ExternalInput")
    bredrep = nc.dram_tensor("bredrep", [128, 16 * ES], f32, kind="ExternalInput")
    w2 = nc.dram_tensor("w2", [128, ES * E], bf, kind="ExternalInput")
    bred2 = nc.dram_tensor("bred2", [128, 2], f32, kind="ExternalInput")
    wo = nc.dram_tensor("wo", [128, 2 * VS], bf, kind="ExternalInput")
    boutrep = nc.dram_tensor("boutrep", [B, VS], f32, kind="ExternalInput")
    ones = nc.dram_tensor("ones", [128, 1], bf, kind="ExternalInput")
    out_ext = nc.dram_tensor("out", [B, VS], f32, kind="ExternalOutput")

    # gather buffer: h section in hdt, s row in bf16 (aliased view when fp8)
    if H_FP8:
        glen = GHT + 2 * NCOL          # e3m4 elements (s = 512 bf16 = 1024 bytes)
        gin = nc.dram_tensor("gin", [glen], hdt)
        gout = nc.dram_tensor("gout", [NC, glen], hdt, addr_space="Shared")
        gin_s = type(gin)("gin", [GHT // 2 + NCOL], bf)
        gout_s = type(gout)("gout", [NC, GHT // 2 + NCOL], bf)
        s_off = GHT // 2
    else:
        glen = GLEN
        gin = nc.dram_tensor("gin", [glen], bf)
        gout = nc.dram_tensor("gout", [NC, glen], bf, addr_space="Shared")
        gin_s, gout_s, s_off = gin, gout, GHT
    y2g_in = nc.dram_tensor("y2g_in", [B, E], f32)
    y2g_out = nc.dram_tensor("y2g_out", [B, E], f32, addr_space="Shared")

    groups = [list(range(NC))]

    with tile.TileContext(nc) as tc:
        with (
            tc.tile_pool(name="persist", bufs=1) as pp,
            tc.tile_pool(name="xload", bufs=3) as xpool,
            tc.tile_pool(name="psum", bufs=1, space="PSUM") as psp,
        ):
            # ---------- persistent SBUF ----------
            wemb_res = pp.tile([128, NV * E], bf)        # [v128, (vc, e)]
            wr_res = pp.tile([128, ES * E], bf)          # [k, (el, j)]
            w2_res = pp.tile([128, ES * E], bf)          # [k, (jl, eo)]
            wo_sb = pp.tile([128, 2 * VS], bf)           # [e128, (ec, v)]
            hT_all = pp.tile([128, 2 * B * S], bf)       # [j128, (jc, b, s)]
            sT_all = pp.tile([128, B], bf)               # [k, (c, bl)]
            weff = pp.tile([128, 2 * ES * B], bf)        # [j128, (jc, e, b)]
            y1 = pp.tile([128, B * ES], bf)              # [k, (b, j)]
            hsb = pp.tile([128, 2 * NCOL], hdt)          # local hT [j128, (jc, n)]
            s_bf = pp.tile([1, NCOL], bf)
            bembe_sb = pp.tile([128, 2], f32)
            bredrep_sb = pp.tile([128, 16 * ES], f32)
            bred2_sb = pp.tile([128, 2], f32)
            ones_sb = pp.tile([128, 1], bf)
            y2sum = pp.tile([B, E], f32)
            y2Tf = pp.tile([128, 2 * B], f32)            # [e128, (ec, b)]
            y2T = pp.tile([128, 2 * B], bf)
            boutrep_sb = pp.tile([B, VS], f32)
            outsb = pp.tile([B, VS], f32)

            # tiny biases first on the scalar (ACT) HWDGE ring
            nc.scalar.dma_start(bembe_sb[:, :], bembe[:, :])
            nc.scalar.dma_start(bred2_sb[:, :], bred2[:, :])
            nc.scalar.dma_start(ones_sb[:, :], ones[:, :])

            weff_v = weff.rearrange("p (jc e b) -> p jc e b", jc=2, e=ES)
            weff_p = weff.rearrange("p (jc e b) -> p e jc b", jc=2, e=ES)
            y1_v = y1.rearrange("p (b j) -> p b j", b=B)

            _ORD = ["p1", "gather", "p3", "p4", "p5", "ar", "tail1", "tail2", "all"]

            def upto(stage):
                return _ORD.index(stage) <= _ORD.index(stop_after)

            for _rep in range(reps):
                # ------- phase 1: hT = relu(w_embT.T @ (x-.5)T + bembe) -------
                ph0 = psp.tile([128, NCOL], f32, tag="ph0", name="ph0")
                ph1 = psp.tile([128, NCOL], f32, tag="ph1", name="ph1")
                ps = psp.tile([1, NCOL], f32, tag="ps", name="ps")
                phs = [ph0, ph1]
                for g in range(NV // XCH):
                    xt_t = xpool.tile([128, XCH * NCOL], xdt, tag="xt", name="xt_t")
                    nc.sync.dma_start(
                        xt_t[:, :], xt[:, g * XCH * NCOL:(g + 1) * XCH * NCOL]
                    )
                    nc.scalar.dma_start(
                        wemb_res[:, g * XCH * E:(g + 1) * XCH * E],
                        wemb[:, g * XCH * E:(g + 1) * XCH * E],
                    )
                    for vci in range(XCH):
                        vc = g * XCH + vci
                        for ec in range(2):
                            nc.tensor.matmul(
                                phs[ec][:, :],
                                wemb_res[:, vc * E + ec * 128: vc * E + (ec + 1) * 128],
                                xt_t[:, vci * NCOL:(vci + 1) * NCOL],
                                start=(vc == 0),
                                stop=(vc == NV - 1),
                            )
                for ec in range(2):
                    nc.scalar.activation(
                        hsb[:, ec * NCOL:(ec + 1) * NCOL],
                        phs[ec][:, :],
                        AF.Relu,
                        bias=bembe_sb[:, ec:ec + 1],
                    )
                # s = column sums of hT (over both j-chunks)
                for ec in range(2):
                    nc.tensor.matmul(
                        ps[:, :],
                        ones_sb[:, 0:1],
                        hsb[:, ec * NCOL:(ec + 1) * NCOL],
                        start=(ec == 0),
                        stop=(ec == 1),
                    )
                nc.vector.tensor_copy(s_bf[:, :], ps[:, :])

                if not upto("gather"):
                    nc.gpsimd.dma_start(out_ext[:, 0:NCOL], hsb[0:B, 0:NCOL])
                    continue
                # ------- single merged AllGather of (hT, s) -------
                nc.sync.dma_start(
                    gin.ap()[0:GHT].rearrange("(jc p n) -> p jc n", jc=2, p=128),
                    hsb.rearrange("p (jc n) -> p jc n", jc=2),
                )
                nc.sync.dma_start(
                    gin_s.ap()[s_off:s_off + NCOL].rearrange("(one n) -> one n",
                                                             one=1),
                    s_bf[:, :],
                )
                if skip_cc:
                    nc.sync.dma_start(gout.ap()[0], gin.ap()[:])
                else:
                    nc.gpsimd.collective_compute(
                        "AllGather", ALU.bypass, groups,
                        ins=[gin.ap().opt()], outs=[gout.ap().opt()],
                    )
                # steps 3-5 weights stream on the ACT ring during the gather
                nc.scalar.dma_start(wr_res[:, 0:ES * E // 2], wr[:, 0:ES * E // 2])
                nc.scalar.dma_start(wr_res[:, ES * E // 2:], wr[:, ES * E // 2:])
                nc.scalar.dma_start(bredrep_sb[:, :], bredrep[:, :])
                nc.scalar.dma_start(w2_res[:, 0:ES * E // 2], w2[:, 0:ES * E // 2])
                nc.scalar.dma_start(w2_res[:, ES * E // 2:], w2[:, ES * E // 2:])
                nc.scalar.dma_start(
                    wo_sb.rearrange("p (ec v) -> p ec v", ec=2),
                    wo.ap().rearrange("p (ec v) -> p ec v", ec=2),
                )
                nc.scalar.dma_start(boutrep_sb[:, :], boutrep[:, :])

                # readback: hT of all cores + s rows
                # readback: sT first (step 3 needs it before hT is needed)
                sT_v = sT_all.rearrange("p (c bl) -> p c bl", c=NC)
                g_s = gout_s.ap()[:, s_off:s_off + NCOL].rearrange(
                    "c (bl k) -> k c bl", bl=BL
                )
                for bl in range(BL):
                    nc.sync.dma_start(sT_v[:, :, bl], g_s[:, :, bl])
                h_dma = nc.gpsimd.dma_start if H_FP8 else nc.sync.dma_start
                for jc in range(2):
                    h_dma(
                        hT_all[:, jc * B * S:(jc + 1) * B * S].rearrange(
                            "p (c n) -> p c n", c=NC
                        ),
                        gout.ap()[:, jc * 128 * NCOL:(jc + 1) * 128 * NCOL].rearrange(
                            "c (p n) -> p c n", p=128
                        ),
                    )

                if not upto("p3"):
                    nc.gpsimd.dma_start(out_ext[:, 0:NCOL], hT_all[0:B, 0:NCOL])
                    nc.gpsimd.dma_start(out_ext[:, NCOL:NCOL + B], sT_all[0:B, :])
                    continue
                # ------- step 3: Weff[b, e, j] = sum_k Wr[e,k,j] s[b,k] -------
                for g in range(ES // 8):
                    pw8 = psp.tile([128, 512], f32, tag="p32", bufs=2, name="pw8")
                    for e8 in range(8):
                        el = g * 8 + e8
                        for jc in range(2):
                            off = e8 * 64 + jc * 32
                            nc.tensor.matmul(
                                pw8[:, off:off + 32],
                                wr_res[:, el * E + jc * 128: el * E + (jc + 1) * 128],
                                sT_all[:, :],
                            )
                    nc.vector.tensor_copy(
                        weff_p[:, g * 8:(g + 1) * 8, :, :],
                        pw8.rearrange("p (e jc b) -> p e jc b", e=8, jc=2),
                    )

                if not upto("p4"):
                    nc.gpsimd.dma_start(out_ext[:, 0:64], weff[0:B, 0:64])
                    nc.gpsimd.dma_start(out_ext[:, 64:64 + NCOL], hT_all[0:B, 0:NCOL])
                    continue
                # ------- step 4: y1 = relu(h @ Weff_b^T + b_red) -------
                for g in range(B // 16):
                    py16 = psp.tile([128, 512], f32, tag="p32", bufs=2, name="py16")
                    for bl in range(16):
                        b = g * 16 + bl
                        for jc in range(2):
                            nc.tensor.matmul(
                                py16[:, bl * ES:(bl + 1) * ES],
                                hT_all[:, jc * B * S + b * S: jc * B * S + (b + 1) * S],
                                weff_v[:, jc, :, b],
                                start=(jc == 0),
                                stop=(jc == 1),
                            )
                    nc.vector.tensor_tensor(py16[:, :], py16[:, :], bredrep_sb[:, :], ALU.add)
                    nc.scalar.activation(
                        y1[:, g * 512:(g + 1) * 512], py16[:, :], AF.Relu
                    )

                if not upto("p5"):
                    nc.gpsimd.dma_start(out_ext[:, 0:64], y1[0:B, 0:64])
                    continue
                # ------- step 5: y2p[b, eo] = sum_{k, j in shard} -------
                py2 = psp.tile([B, E], f32, tag="py2", name="py2")
                for jl in range(ES):
                    nc.tensor.matmul(
                        py2[:, :],
                        y1_v[:, :, jl],
                        w2_res[:, jl * E:(jl + 1) * E],
                        start=(jl == 0),
                        stop=(jl == ES - 1),
                    )
                nc.vector.tensor_copy(y2sum[:, 0:E], py2[:, :])

                if not upto("ar"):
                    nc.sync.dma_start(out_ext[:, 0:E], y2sum[:, 0:E])
                    continue
                # ------- AllReduce partial y2 -------
                nc.sync.dma_start(y2g_in[:, :], y2sum[:, 0:E])
                if skip_cc:
                    nc.sync.dma_start(y2g_out[:, :], y2g_in[:, :])
                else:
                    nc.gpsimd.collective_compute(
                        "AllReduce", ALU.add, groups,
                        ins=[y2g_in.ap().opt()], outs=[y2g_out.ap().opt()],
                    )
                nc.sync.dma_start(y2sum[:, 0:E], y2g_out[:, :])

                if not upto("tail1"):
                    nc.gpsimd.dma_start(out_ext[:, 0:E], y2sum[:, 0:E])
                    continue

                # ------- y2T = relu(y2 + b_red2)^T (DVE 32-blocks) -------
                for ec in range(2):
                    for j4 in range(4):
                        nc.vector.transpose(
                            y2Tf[j4 * 32:(j4 + 1) * 32, ec * B:(ec + 1) * B],
                            y2sum[:, ec * 128 + j4 * 32: ec * 128 + (j4 + 1) * 32],
                        )
                    nc.scalar.activation(
                        y2T[:, ec * B:(ec + 1) * B],
                        y2Tf[:, ec * B:(ec + 1) * B],
                        AF.Relu,
                        bias=bred2_sb[:, ec:ec + 1],
                    )

                if not upto("tail2"):
                    nc.gpsimd.dma_start(out_ext[:, 0:2 * B], y2T[0:B, :])
                    continue
                # ------- step 6: out = y2 @ w_out^T + b_out -------
                for nv in range(2):
                    pso = psp.tile([B, 512], f32, tag="po", bufs=2, name="pso")
                    for ec in range(2):
                        nc.tensor.matmul(
                            pso[:, :],
                            y2T[:, ec * B:(ec + 1) * B],
                            wo_sb[:, ec * VS + nv * 512: ec * VS + (nv + 1) * 512],
                            start=(ec == 0),
                            stop=(ec == 1),
                        )
                    nc.vector.tensor_tensor(
                        outsb[:, nv * 512:(nv + 1) * 512],
                        pso[:, :],
                        boutrep_sb[:, nv * 512:(nv + 1) * 512],
                        ALU.add,
                    )
                nc.sync.dma_start(out_ext[:, :], outsb[:, :])

    nc.compile()
    return nc


def _get_nc():
    if "nc" not in _CACHE:
        _CACHE["nc"] = _build_nc()
    return _CACHE["nc"]


def _pm(a):
    """[V-like rows, cols] -> partition-major [128, (chunks, cols)]."""
    v, c = a.shape
    return np.ascontiguousarray(
        a.reshape(v // 128, 128, c).transpose(1, 0, 2).reshape(128, -1)
    )


def _pack_inputs(x, w_emb, b_emb, w_red, b_red, w_red2, b_red2, w_out, b_out):
    bf = ml_dtypes.bfloat16
    xdt = ml_dtypes.float8_e3m4 if X_FP8 else bf
    f32 = np.float32

    x = np.asarray(x, f32)
    w_emb = np.asarray(w_emb, f32)
    wembT = _pm(np.ascontiguousarray(w_emb.T)).astype(bf)            # [128,(vc,e)]
    bemb_eff = (np.asarray(b_emb, np.float64)
                + 0.5 * np.asarray(w_emb, np.float64).sum(axis=1)).astype(f32)
    bembe = np.ascontiguousarray(bemb_eff.reshape(2, 128).T)         # [128, 2]
    Wr = np.asarray(w_red, f32).reshape(E, S, E)                     # [e, k, j]
    W2 = np.asarray(w_red2, f32).reshape(E, S, E)                    # [eo, k, j]
    woT = np.ascontiguousarray(np.asarray(w_out, f32).T)             # [E, V]
    bred2c = np.ascontiguousarray(
        np.asarray(b_red2, f32).reshape(2, 128).T)                   # [128, 2]
    ones = np.ones((128, 1), dtype=bf)

    in_maps = []
    for c in range(NC):
        xs = np.asarray(x[c * BL:(c + 1) * BL])                      # [4, S, V]
        xc = xs.transpose(2, 0, 1).reshape(V, NCOL) - 0.5            # [V, 512]
        xt = _pm(xc).astype(xdt)                                     # [128,(vc,n)]
        # wr: [k, (el, j)]
        wr_c = np.ascontiguousarray(
            Wr[c * ES:(c + 1) * ES].transpose(1, 0, 2).reshape(S, ES * E)
        ).astype(bf)
        # w2: [k, (jl, eo)]
        w2_c = np.ascontiguousarray(
            W2[:, :, c * ES:(c + 1) * ES].transpose(1, 2, 0).reshape(S, ES * E)
        ).astype(bf)
        # wo: [e128, (ec, v)]
        wo_c = _pm(woT[:, c * VS:(c + 1) * VS]).astype(bf)           # [128,(ec,v)]
        bredrep = np.ascontiguousarray(
            np.broadcast_to(np.tile(b_red[c * ES:(c + 1) * ES], 16), (S, 16 * ES))
        ).astype(f32)
        boutrep = np.ascontiguousarray(
            np.broadcast_to(b_out[c * VS:(c + 1) * VS], (B, VS))
        ).astype(f32)
        in_maps.append({
            "xt": xt, "wemb": wembT, "bembe": bembe,
            "wr": wr_c, "bredrep": bredrep,
            "w2": w2_c, "bred2": bred2c,
            "wo": wo_c, "boutrep": boutrep,
            "ones": ones,
        })
    return in_maps


def kernel(x, w_emb, b_emb, w_red, b_red, w_red2, b_red2, w_out, b_out):
    from concourse.bass_utils import run_bass_kernel_spmd

    nc = _get_nc()
    x, w_emb, b_emb, w_red, b_red, w_red2, b_red2, w_out, b_out = (
        np.asarray(a, dtype=np.float32)
        for a in (x, w_emb, b_emb, w_red, b_red, w_red2, b_red2, w_out, b_out)
    )
    in_maps = _pack_inputs(x, w_emb, b_emb, w_red, b_red, w_red2, b_red2, w_out, b_out)
    res = run_bass_kernel_spmd(nc, in_maps, core_ids=list(range(NC)))
    out = np.concatenate([res.results[c]["out"] for c in range(NC)], axis=1)
    return np.ascontiguousarray(out, dtype=np.float32)
